# revision 15
# baseline (speedup 1.0000x reference)
"""Trainium2 Bass kernel for nn_AttentionAugmentation.

Attention with 2D relative-position logits. B=8, H=W=32, dk=dv=256, Nh=8.
Sharding: data-parallel over batch (one batch per NeuronCore, 8 cores).

Per-core v3 (one batch, 8 heads of 1024x1024 attention, dkh=32):
  - fp16 datapath end-to-end (inputs cast fp32->fp16 in the SWDGE DMA):
    q/k/v, qaug/kaug, rel keys, wexp are all fp16 -- higher precision than
    bf16 buys error budget for the cheap DVE exp below.
  - qT/kT via PE transposes -> cast -> partition-scatter DMAs into
    qaug/kaug rows 0-31. dk^-0.5 folded into the exp (scale / Schraudolph A).
  - rel logits in a 96-row augmented contraction: kaug rows 32-63 one-hot
    of key y2, rows 64-95 one-hot of key x2; qaug rows 32-63 = WRELT,
    rows 64-95 = HRELT, computed by shifted krw/krh^T-window matmuls.
    Rel psum is copied STRAIGHT to qaug with strided 1x DVE/ACT copies
    (runs-of-1 for W; runs-of-32 for H) -- no staging, no GPSIMD scatter.
  - attention per head-pair, software-pipelined per 128-key chunk:
    S^T = QK matmuls (f32 psum); exp split between ACT (scalar.activation
    Exp -> fp16) and a DVE fp16 pair-product Schraudolph: one
    tensor_scalar makes i1 = int16(A*logit + B1); GPSIMD shifts
    i2 = i1 - 512 (half-period stagger); one DVE tensor_tensor multiplies
    the two int16-bitcast-fp16 staircases => exp within ~1% with ~zero
    mean bias (C=58 debias), so mixing with ACT chunks is safe.
    AV uses lhsT=[V | 1] per head; the two heads of a pair write att psum
    partitions 0-32 / 64-96 (col-tiled concurrent matmuls).
  - output per pair, pipelined per 512-col half: att psum -> fp16 SBUF
    (x2^-6; ACT half / DVE half in parallel), xbar DMA-transpose per half,
    reciprocal per half, then per-(head,chunk) reciprocal-scaled copies
    into out_sb (GPSIMD for pairs 0-2; DVE/ACT split on the last pair to
    shorten the tail), and a per-pair DMA of the 64 output channels.
"""
import sys

sys.path.insert(0, "/opt/trn_rl_repo")

from contextlib import ExitStack

import numpy as np

import concourse.bass as bass
from concourse import bacc
import concourse.mybir as mybir
from concourse import masks
from concourse.tile import TileContext

HW = 1024
CH = 768
NH = 8
F32 = mybir.dt.float32
FP16 = mybir.dt.float16
I16 = mybir.dt.int16
EXP = mybir.ActivationFunctionType.Exp
COPY = mybir.ActivationFunctionType.Copy
MULT = mybir.AluOpType.mult
ADD = mybir.AluOpType.add
QSCALE = float((256 / 8) ** -0.5)
# fp16 pair-product Schraudolph: i1 = int16(A*x + B1), i2 = i1 - 512,
# exp(x*QSCALE) ~= fp16(i1) * fp16(i2). A folds QSCALE; C=58 zeroes the
# mean bias so ACT-exp and DVE-exp chunks can mix inside one softmax row.
SCH_A = 512.0 * QSCALE / np.log(2.0)
SCH_B1 = 15360.0 + 256.0 - 58.0
# (hh, c) chunks computed on the DVE path, per pair index.
DVE_CHUNKS = {
    0: {(0, 4), (0, 6), (1, 5), (1, 7)},
    1: {(0, 1), (0, 4), (0, 6), (1, 2), (1, 5), (1, 7)},
    2: {(0, 1), (0, 4), (0, 6), (1, 2), (1, 5), (1, 7)},
    3: {(0, 1), (0, 4), (0, 6), (1, 2), (1, 5), (1, 7)},
}


def build_nc():
    nc = bacc.Bacc()
    # input split in two halves: a single [1024, 768] parameter makes the
    # axon-pjrt reshard program's dynamic-slice exceed a 16-bit semaphore
    # field in neuronx-cc (25MB concat across 8 cores), crashing walrus.
    xa_d = nc.declare_dram_parameter("xa", [HW // 2, CH], F32, isOutput=False)
    xb_d = nc.declare_dram_parameter("xb", [HW // 2, CH], F32, isOutput=False)
    krw_d = nc.declare_dram_parameter("krw", [63, 32], F32, isOutput=False)
    krh_d = nc.declare_dram_parameter("krh", [63, 32], F32, isOutput=False)
    out_d = nc.declare_dram_parameter("out", [HW, 256], F32, isOutput=True)

    with ExitStack() as octx:
        tc = octx.enter_context(TileContext(nc))
        sb = octx.enter_context(tc.tile_pool(name="persist", bufs=1))

        x_sb = sb.tile([128, 8 * CH], FP16)     # natural input: part p, col 768c+ch
        qaug = sb.tile([96, NH * HW], FP16)     # per head h: cols 1024h + (32x + y)
        kaug = sb.tile([96, NH * HW], FP16)
        v1 = sb.tile([128, NH * 8 * 33], FP16)  # per (h,c): 33 cols = V chunk | ones
        tscr = sb.tile([128, 2 * HW], FP16)     # transpose scratch (2 groups live)
        wnat = sb.tile([64, NH * HW], FP16)     # rows 32-63: W rel, (y, h, x) major
        out_sb = sb.tile([128, 8 * 256], F32)   # col 256c + ch
        identb = sb.tile([128, 128], FP16)
        krw_sb = sb.tile([63, 32], FP16)
        krh_sb = sb.tile([63, 32], FP16)
        # zero-padded transposed rel keys: walrus rejects 32-contraction
        # matmuls whose psum out starts at partition 32/64, so the rel MMs
        # use wider lhsT windows that land the useful rows at 32-63 (W) /
        # 64-95 (H) of a base-0 psum tile instead.
        krwT = sb.tile([32, 128], FP16)   # krwT[:, 32+m] = krw^T[:, m]
        krhT = sb.tile([32, 160], FP16)   # krhT[:, 64+m] = krh^T[:, m]

        # ---- constants first: identity lands fast so the PE warm-up can
        # start while the input DMAs stream ----
        masks.make_identity(nc, identb[:])

        # ---- input DMAs (SWDGE: fp32 -> fp16 cast). krw/krh go first (tiny
        # transfers; the SWDGE queue is FIFO and anything after 3MB of x
        # would land ~20us in). x loads q cols first. ----
        nc.gpsimd.dma_start(out=krw_sb[:], in_=krw_d[:])
        nc.gpsimd.dma_start(out=krh_sb[:], in_=krh_d[:])
        xv = x_sb[:].rearrange("p (c g) -> p c g", c=8, g=768)
        for col0 in (0, 256, 512):              # q, k, v column groups
            for half, src_d in ((0, xa_d), (1, xb_d)):
                nc.gpsimd.dma_start(
                    out=xv[:, 4 * half:4 * half + 4, col0:col0 + 256],
                    in_=src_d[:].rearrange("(c p) g -> p c g", p=128)
                        [:, :, col0:col0 + 256],
                )
        # one-hot rows of kaug, head-0 block only: rows 32-63: [y2(k)==j],
        # rows 64-95: [x2(k)==j]; col = 32*x2 + y2. Then DMA-replicate to
        # the other 7 head blocks (log-doubling).
        nc.gpsimd.memset(kaug[32:64, 0:HW], 0.0)
        nc.gpsimd.memset(kaug[64:96, 0:HW], 0.0)
        nc.gpsimd.affine_select(
            out=kaug[32:64, 0:HW].rearrange("p (x y) -> p x y", x=32, y=32),
            in_=kaug[32:64, 0:HW].rearrange("p (x y) -> p x y", x=32, y=32),
            compare_op=mybir.AluOpType.not_equal,
            fill=1.0,
            base=0,
            pattern=[[0, 32], [-1, 32]],
            channel_multiplier=1,
        )
        nc.gpsimd.affine_select(
            out=kaug[64:96, 0:HW].rearrange("p (x y) -> p x y", x=32, y=32),
            in_=kaug[64:96, 0:HW].rearrange("p (x y) -> p x y", x=32, y=32),
            compare_op=mybir.AluOpType.not_equal,
            fill=1.0,
            base=0,
            pattern=[[-1, 32], [0, 32]],
            channel_multiplier=1,
        )
        n = HW
        while n < NH * HW:
            rep = min(n, NH * HW - n)
            nc.sync.dma_start(
                out=kaug[32:96, n:n + rep],
                in_=kaug[32:96, 0:rep],
            )
            n += rep
        # V1: ones only in col 32 of each 33-block; V chunks fill cols 0-31
        # (copies emitted below, on GPSIMD behind the pair-0 W scatter).
        v1v = v1[:].rearrange("p (h c e) -> p h c e", h=8, c=8, e=33)
        nc.gpsimd.memset(v1v[:, :, :, 32], 1.0)
        v1c = v1[:].rearrange("p (h c e) -> p c h e", h=8, c=8, e=33)

        # ================= Phase A: transposes + rel logits =================
        with ExitStack() as actx:
            psA = actx.enter_context(tc.tile_pool(name="psA", bufs=2, space="PSUM"))
            psR = actx.enter_context(tc.tile_pool(name="psR", bufs=2, space="PSUM"))

            # PE warm-up: back-to-back REAL matmuls so the HAM clock gate
            # opens (1.2 -> 2.4 GHz) while the input DMA streams in.
            # Transpose-mode does NOT count as PE-busy for HAM, so the
            # warm-up must be plain matmuls (~3.5us of sustained activity).
            wps = psA.tile([128, 512], F32, tag="warm")
            for i in range(32):
                nc.tensor.matmul(
                    out=wps[:, 0:128], lhsT=identb[:], rhs=identb[:],
                    start=True, stop=True,
                )
            nc.vector.memset(krwT[:], 0.0)
            nc.vector.memset(krhT[:], 0.0)

            # qT: PE-transpose 4-head groups, cast, partition-scatter
            # DMAs into qaug rows 0-31. (k groups done after w_pair(0).)
            def transpose_group(kind, g, dstt):
                col0 = 256 * kind + 128 * g
                pt = psA.tile([128, HW], FP16, tag="tps")
                for c in range(8):
                    nc.tensor.transpose(
                        out=pt[:, 128 * c:128 * c + 128],
                        in_=x_sb[:, 768 * c + col0:768 * c + col0 + 128],
                        identity=identb[:, 0:128],
                    )
                scr = tscr[:, HW * g:HW * g + HW]
                # casts split across DVE/ACT so the two groups overlap
                if g == 0:
                    nc.vector.tensor_copy(out=scr, in_=pt[:])
                else:
                    nc.scalar.copy(out=scr, in_=pt[:])
                for hh in range(4):
                    h = 4 * g + hh
                    dma_eng = nc.sync if hh % 2 == 0 else nc.scalar
                    dma_eng.dma_start(
                        out=dstt[0:32, HW * h:HW * h + HW],
                        in_=tscr[32 * hh:32 * hh + 32, HW * g:HW * g + HW],
                    )

            for g in range(2):
                transpose_group(0, g, qaug)

            # key_rel transposes: krw [63,32] -> krwT cols 32-94 (zero pad).
            for srct, dst, off in ((krw_sb, krwT, 32), (krh_sb, krhT, 64)):
                pt = psA.tile([128, HW], FP16, tag="tps")
                nc.tensor.transpose(
                    out=pt[0:32, 0:63], in_=srct[:], identity=identb[0:63, 0:63]
                )
                nc.vector.tensor_copy(out=dst[0:32, off:off + 63], in_=pt[0:32, 0:63])

            qa4 = qaug[0:32, :].rearrange("p (h x y) -> p h x y", h=8, x=32, y=32)
            hdst = qaug[64:96, :].rearrange("p (h x y) -> p h x y", h=8, x=32, y=32)
            wdst = qaug[32:64, :].rearrange("p (h x y) -> p h x y", h=8, x=32, y=32)
            # wnat per head-pair block: cols 8192p + (i32, h2, x32)
            wnp = wnat[32:64, :].rearrange(
                "p (pp i h x) -> p pp i h x", pp=4, i=32, h=2, x=32)

            def w_half(p, ss, eng, pool, tag):
                # W rel for heads 2p,2p+1, y=i in [16ss, 16ss+16): one MM
                # (N=64) per i; staged contiguously (i,h,x) on DVE/ACT.
                # (runs-of-1 strided copies cost ~4.7us on DVE/ACT --
                # measured -- so the final scatter stays on GPSIMD.)
                pw = pool.tile([96, HW], F32, tag=tag)
                pwmm = pw[0:64, :].rearrange(
                    "p (i h x) -> p i h x", i=16, h=2, x=32)
                for ii in range(16):
                    i = 16 * ss + ii
                    nc.tensor.matmul(
                        out=pwmm[:, ii, :, :],
                        lhsT=krwT[:, 31 - i:95 - i],
                        rhs=qa4[:, 2 * p:2 * p + 2, :, i],
                        start=True, stop=True,
                    )
                src = pw[32:64, :].rearrange(
                    "p (i h x) -> p i h x", i=16, h=2, x=32)
                dst = wnp[:, p, 16 * ss:16 * ss + 16, :, :]
                if eng == "act":
                    nc.scalar.copy(out=dst, in_=src)
                else:
                    nc.vector.tensor_copy(out=dst, in_=src)

            def w_scatter(p, ss):
                # scatter one staged y-half to qaug rows 32-63 (runs of 1)
                # on GPSIMD cores 2-3, which own partitions 32-63.
                for hh in range(2):
                    h = 2 * p + hh
                    nc.gpsimd.tensor_copy(
                        out=wdst[:, h, :, 16 * ss:16 * ss + 16],
                        in_=wnp[:, p, 16 * ss:16 * ss + 16, hh, :]
                            .rearrange("p i x -> p x i"),
                    )

            def w_pair(p, eng, pool, tag="rel"):
                for ss in range(2):
                    w_half(p, ss, eng, pool, tag)
                for ss in range(2):
                    w_scatter(p, ss)

            def h_group(g):
                # H rel, all heads, x = 4g..4g+3; copies go straight to
                # qaug rows 64-95 (runs of 32), alternating ScalarE/DVE.
                ph = psR.tile([96, HW], F32, tag="rel")
                phmm = ph[:].rearrange("p (i h y) -> p i h y", i=4, h=8, y=32)
                for j in range(4):
                    i = 4 * g + j
                    nc.tensor.matmul(
                        out=phmm[:, j, :, :],
                        lhsT=krhT[:, 31 - i:127 - i],
                        rhs=qa4[:, :, i, :],
                        start=True, stop=True,
                    )
                dst = hdst[:, :, 4 * g:4 * g + 4, :]
                src = ph[64:96, :].rearrange(
                    "p (i h y) -> p h i y", i=4, h=8, y=32)
                if g % 2 == 0:
                    nc.scalar.copy(out=dst, in_=src)
                else:
                    nc.vector.tensor_copy(out=dst, in_=src)

            def v_copy(c):
                nc.gpsimd.tensor_copy(
                    out=v1c[:, c, :, 0:32],
                    in_=x_sb[:, 768 * c + 512:768 * c + 512 + 256]
                        .rearrange("p (h e) -> p h e", h=8),
                )

            # W pair 0, both kT groups, and ALL H groups complete first
            # (every pair needs every H stage); V copies sit behind the
            # pair-0 W scatter on the GPSIMD queue. W pairs 1-3 are emitted
            # inside phase B, overlapped under the previous pair's slots.
            w_pair(0, "act", psR)
            transpose_group(1, 0, kaug)
            transpose_group(1, 1, kaug)
            for g in range(8):
                h_group(g)
            for c in range(8):
                v_copy(c)

        # ================= Phase B: attention per head-pair =================
        with ExitStack() as bctx:
            psS = bctx.enter_context(tc.tile_pool(name="psS", bufs=3, space="PSUM"))
            psT = bctx.enter_context(tc.tile_pool(name="psT", bufs=1, space="PSUM"))
            sbW = bctx.enter_context(tc.tile_pool(name="sbW", bufs=4))
            sbE = bctx.enter_context(tc.tile_pool(name="sbE", bufs=4))
            sbA = bctx.enter_context(tc.tile_pool(name="sbA", bufs=2))
            sbT = bctx.enter_context(tc.tile_pool(name="sbT", bufs=2))
            sbR = bctx.enter_context(tc.tile_pool(name="sbR", bufs=2))

            for hp in range(NH // 2):
                # two heads share one att psum: head 2hp at partitions 0-32,
                # head 2hp+1 at partitions 64-96 (col-tiled concurrent AV).
                # Chunk loop software-pipelined: QK(c)+exp-issue(c), then the
                # pending DVE TT from (c-1), then AV(c-1) -- so the in-order
                # PE never sits behind an exp of its own chunk, and the DVE
                # never idles between TS1 and its TT (GPSIMD shift overlaps).
                att = psT.tile([97, HW], F32, tag="att")
                wexp_prev = None
                pend_tt = []
                for c in range(9):
                    # next pair's W rel, overlapped under this pair's
                    # exp-bound slots: MM+stage halves at c=1/3 (one psS
                    # slot held at a time), scatter halves at c=3/5 so the
                    # GPSIMD finishes before the next pair's first QK.
                    if hp < NH // 2 - 1:
                        eng = "dve" if hp % 2 == 0 else "act"
                        if c == 1:
                            w_half(hp + 1, 0, eng, psS, "sT")
                        elif c == 3:
                            w_half(hp + 1, 1, eng, psS, "sT")
                            w_scatter(hp + 1, 0)
                        elif c == 5:
                            w_scatter(hp + 1, 1)
                    wexps = []
                    if c < 8:
                        for hh in range(2):
                            h = 2 * hp + hh
                            s_ps = psS.tile([128, HW], F32, tag="sT")
                            for e in range(2):
                                nc.tensor.matmul(
                                    out=s_ps[:, 512 * e:512 * e + 512],
                                    lhsT=kaug[:, HW * h + 128 * c:HW * h + 128 * c + 128],
                                    rhs=qaug[:, HW * h + 512 * e:HW * h + 512 * e + 512],
                                    start=True, stop=True,
                                )
                            wexp = sbW.tile([128, HW], FP16, tag="wexp")
                            if (hh, c) in DVE_CHUNKS[hp]:
                                e1 = sbE.tile([128, HW], I16, tag="e1")
                                e2 = sbE.tile([128, HW], I16, tag="e2")
                                nc.vector.tensor_scalar(
                                    out=e1[:], in0=s_ps[:],
                                    scalar1=SCH_A, scalar2=SCH_B1,
                                    op0=MULT, op1=ADD,
                                )
                                nc.vector.tensor_scalar_add(
                                    out=e2[:], in0=e1[:], scalar1=-512.0,
                                )
                                pend_tt.append((e1, e2, wexp))
                            else:
                                nc.scalar.activation(
                                    out=wexp[:], in_=s_ps[:], func=EXP, scale=QSCALE,
                                )
                            wexps.append(wexp)
                    # pending TTs from the previous slot: run on DVE while
                    # this slot's QKs stream on the PE, ahead of AV(c-1).
                    for e1, e2, wexp in pend_tt:
                        nc.vector.tensor_tensor(
                            out=wexp[:],
                            in0=e1[:].bitcast(FP16),
                            in1=e2[:].bitcast(FP16),
                            op=MULT,
                        )
                    pend_tt = []
                    if c > 0:
                        # AV(c-1), e-major: the two heads' matmuls sit in
                        # different PE col groups and run concurrently.
                        for e in range(2):
                            for hh in range(2):
                                h = 2 * hp + hh
                                nc.tensor.matmul(
                                    out=att[64 * hh:64 * hh + 33, 512 * e:512 * e + 512],
                                    lhsT=v1[:, 264 * h + 33 * (c - 1):264 * h + 33 * (c - 1) + 33],
                                    rhs=wexp_prev[hh][:, 512 * e:512 * e + 512],
                                    start=(c - 1 == 0), stop=(c - 1 == 7),
                                )
                    wexp_prev = wexps

                # Output, pipelined per 512-col (e) half:
                # att -> fp16 SBUF (x 2^-6, cancels in the normalization);
                # ACT takes e=0, DVE takes e=1 so the halves overlap and the
                # (single-buffered) att psum frees for the next pair ASAP.
                att_sb = sbA.tile([112, HW], FP16, tag="attsb")
                nc.vector.memset(att_sb[96:112, :], 0.0)
                nc.scalar.activation(
                    out=att_sb[0:97, 0:512], in_=att[:, 0:512],
                    func=COPY, scale=float(2.0 ** -6),
                )
                nc.vector.tensor_scalar_mul(
                    out=att_sb[0:97, 512:1024],
                    in0=att[:, 512:1024],
                    scalar1=float(2.0 ** -6),
                )
                # xbar DMA transpose per half: [112, 512] -> [128, (c4, j112)]
                att_t = sbT.tile([128, 8 * 112], FP16, tag="attt")
                att_tv = att_t[:].rearrange("p (c j) -> p c j", c=8, j=112)
                rc = sbR.tile([128, 16], F32, tag="rc")
                rcv = rc[:].rearrange("p (c h) -> p c h", c=8, h=2)
                last = hp == NH // 2 - 1
                for e in range(2):
                    nc.sync.dma_start_transpose(
                        out=att_tv[:, 4 * e:4 * e + 4, :],
                        in_=att_sb[:, 512 * e:512 * e + 512],
                    )
                    # reciprocal of the denominators (row 32 / 96 of att)
                    nc.vector.reciprocal(
                        out=rcv[:, 4 * e:4 * e + 4, :],
                        in_=att_tv[:, 4 * e:4 * e + 4, 32:97:64],
                    )
                    for cc in range(4):
                        cg = 4 * e + cc
                        for hh in range(2):
                            h = 2 * hp + hh
                            dst = out_sb[:, 256 * cg + 32 * h:256 * cg + 32 * h + 32]
                            src = att_tv[:, cg, 64 * hh:64 * hh + 32]
                            sc = rc[:, 2 * cg + hh:2 * cg + hh + 1]
                            # ACT per-op overhead is ~2.7x DVE's here; on the
                            # last pair every scale is tail-serial, so keep
                            # them all on the (faster per-op) DVE.
                            if hh == 0 or last:
                                nc.vector.tensor_scalar_mul(
                                    out=dst, in0=src, scalar1=sc)
                            else:
                                nc.scalar.activation(
                                    out=dst, in_=src, func=COPY, scale=sc)
                # stream this pair's output columns to DRAM
                nc.sync.dma_start(
                    out=out_d[:].rearrange("(c p) d -> p c d", p=128)
                        [:, :, 64 * hp:64 * hp + 64],
                    in_=out_sb[:].rearrange("p (c d) -> p c d", c=8)
                        [:, :, 64 * hp:64 * hp + 64],
                )
    if not nc.is_finalized():
        nc.finalize()
    return nc


_NC = None


def _ensure_axon_hooks_module():
    """bass_utils imports antenv.axon_hooks unconditionally when trace=True;
    this image's antenv lacks it. Provide a stub so tracing degrades to
    no-trace instead of crashing (a real hook can be set by a profiler)."""
    import types

    if "antenv.axon_hooks" in sys.modules:
        return
    try:
        import antenv.axon_hooks  # noqa: F401
        return
    except ImportError:
        pass
    try:
        import antenv
    except ImportError:
        return
    m = types.ModuleType("antenv.axon_hooks")
    m._hook = None
    m.get_axon_ntff_profile_hook = lambda: m._hook
    m.set_axon_ntff_profile_hook = lambda h: setattr(m, "_hook", h)
    sys.modules["antenv.axon_hooks"] = m
    antenv.axon_hooks = m


def kernel(**inputs):
    global _NC
    x = np.ascontiguousarray(np.asarray(inputs["inputs"], dtype=np.float32))
    krw = np.ascontiguousarray(np.asarray(inputs["key_rel_w"], dtype=np.float32))
    krh = np.ascontiguousarray(np.asarray(inputs["key_rel_h"], dtype=np.float32))
    assert x.shape == (8, 32, 32, 768), x.shape
    assert int(inputs["dk"]) == 256 and int(inputs["dv"]) == 256
    assert int(inputs["Nh"]) == 8

    if _NC is None:
        _NC = build_nc()
    _ensure_axon_hooks_module()
    from concourse.bass_utils import run_bass_kernel_spmd

    in_maps = [
        {
            "xa": x[b].reshape(HW, CH)[:HW // 2],
            "xb": x[b].reshape(HW, CH)[HW // 2:],
            "krw": krw,
            "krh": krh,
        }
        for b in range(8)
    ]
    res = run_bass_kernel_spmd(_NC, in_maps, list(range(8)))
    kernel.last_result = res
    out = np.stack([res.results[b]["out"].reshape(32, 32, 256) for b in range(8)], 0)
    return out


if __name__ == "__main__":
    nc = build_nc()
    print("built ok")


# revision 17
# speedup vs baseline: 1.2162x; 1.2162x over previous
"""Trainium2 Bass kernel for nn_AttentionAugmentation.

Attention with 2D relative-position logits. B=8, H=W=32, dk=dv=256, Nh=8.
Sharding: data-parallel over batch (one batch per NeuronCore, 8 cores).

Per-core v3 (one batch, 8 heads of 1024x1024 attention, dkh=32):
  - fp16 datapath end-to-end (inputs cast fp32->fp16 in the SWDGE DMA):
    q/k/v, qaug/kaug, rel keys, wexp are all fp16 -- higher precision than
    bf16 buys error budget for the cheap DVE exp below.
  - qT/kT via PE transposes -> cast -> partition-scatter DMAs into
    qaug/kaug rows 0-31. dk^-0.5 folded into the exp (scale / Schraudolph A).
  - rel logits in a 96-row augmented contraction: kaug rows 32-63 one-hot
    of key y2, rows 64-95 one-hot of key x2; qaug rows 32-63 = WRELT,
    rows 64-95 = HRELT, computed by shifted krw/krh^T-window matmuls.
    Rel psum is copied STRAIGHT to qaug with strided 1x DVE/ACT copies
    (runs-of-1 for W; runs-of-32 for H) -- no staging, no GPSIMD scatter.
  - attention per head-pair, software-pipelined per 128-key chunk:
    S^T = QK matmuls (f32 psum); exp split between ACT (scalar.activation
    Exp -> fp16) and a DVE fp16 pair-product Schraudolph: one
    tensor_scalar makes i1 = int16(A*logit + B1); GPSIMD shifts
    i2 = i1 - 512 (half-period stagger); one DVE tensor_tensor multiplies
    the two int16-bitcast-fp16 staircases => exp within ~1% with ~zero
    mean bias (C=58 debias), so mixing with ACT chunks is safe.
    AV uses lhsT=[V | 1] per head; the two heads of a pair write att psum
    partitions 0-32 / 64-96 (col-tiled concurrent matmuls).
  - output per pair, pipelined per 512-col half: att psum -> fp16 SBUF
    (x2^-6; ACT half / DVE half in parallel), xbar DMA-transpose per half,
    reciprocal per half, then per-(head,chunk) reciprocal-scaled copies
    into out_sb (GPSIMD for pairs 0-2; DVE/ACT split on the last pair to
    shorten the tail), and a per-pair DMA of the 64 output channels.
"""
import sys

sys.path.insert(0, "/opt/trn_rl_repo")

from contextlib import ExitStack

import numpy as np

import concourse.bass as bass
from concourse import bacc
import concourse.mybir as mybir
from concourse import masks
from concourse.tile import TileContext

HW = 1024
CH = 768
NH = 8
F32 = mybir.dt.float32
FP16 = mybir.dt.float16
I16 = mybir.dt.int16
EXP = mybir.ActivationFunctionType.Exp
COPY = mybir.ActivationFunctionType.Copy
MULT = mybir.AluOpType.mult
ADD = mybir.AluOpType.add
QSCALE = float((256 / 8) ** -0.5)
# fp16 pair-product Schraudolph: i1 = int16(A*x + B1), i2 = i1 - 512,
# exp(x*QSCALE) ~= fp16(i1) * fp16(i2). A folds QSCALE; C=58 zeroes the
# mean bias so ACT-exp and DVE-exp chunks can mix inside one softmax row.
SCH_A = 512.0 * QSCALE / np.log(2.0)
SCH_B1 = 15360.0 + 256.0 - 58.0
# (hh, c) chunks computed on the DVE path, per pair index.
DVE_CHUNKS = {
    0: {(0, 4), (0, 6), (1, 5), (1, 7)},
    1: {(0, 1), (0, 4), (0, 6), (1, 2), (1, 5), (1, 7)},
    2: {(0, 1), (0, 4), (0, 6), (1, 2), (1, 5), (1, 7)},
    3: {(0, 1), (0, 4), (0, 6), (1, 2), (1, 5), (1, 7)},
}


def build_nc():
    nc = bacc.Bacc()
    # input split in two halves: a single [1024, 768] parameter makes the
    # axon-pjrt reshard program's dynamic-slice exceed a 16-bit semaphore
    # field in neuronx-cc (25MB concat across 8 cores), crashing walrus.
    xa_d = nc.declare_dram_parameter("xa", [HW // 2, CH], F32, isOutput=False)
    xb_d = nc.declare_dram_parameter("xb", [HW // 2, CH], F32, isOutput=False)
    krw_d = nc.declare_dram_parameter("krw", [63, 32], F32, isOutput=False)
    krh_d = nc.declare_dram_parameter("krh", [63, 32], F32, isOutput=False)
    out_d = nc.declare_dram_parameter("out", [HW, 256], F32, isOutput=True)

    with ExitStack() as octx:
        tc = octx.enter_context(TileContext(nc))
        sb = octx.enter_context(tc.tile_pool(name="persist", bufs=1))

        x_sb = sb.tile([128, 8 * CH], FP16)     # natural input: part p, col 768c+ch
        qaug = sb.tile([96, NH * HW], FP16)     # per head h: cols 1024h + (32x + y)
        kaug = sb.tile([96, NH * HW], FP16)
        v1 = sb.tile([128, NH * 8 * 33], FP16)  # per (h,c): 33 cols = V chunk | ones
        tscr = sb.tile([128, 2 * HW], FP16)     # transpose scratch (2 groups live)
        wnat = sb.tile([64, NH * HW], FP16)     # rows 32-63: W rel, (y, h, x) major
        out_sb = sb.tile([128, 8 * 256], F32)   # col 256c + ch
        identb = sb.tile([128, 128], FP16)
        krw_sb = sb.tile([63, 32], FP16)
        krh_sb = sb.tile([63, 32], FP16)
        # zero-padded transposed rel keys: walrus rejects 32-contraction
        # matmuls whose psum out starts at partition 32/64, so the rel MMs
        # use wider lhsT windows that land the useful rows at 32-63 (W) /
        # 64-95 (H) of a base-0 psum tile instead.
        krwT = sb.tile([32, 128], FP16)   # krwT[:, 32+m] = krw^T[:, m]
        krhT = sb.tile([32, 160], FP16)   # krhT[:, 64+m] = krh^T[:, m]

        # ---- constants first: identity lands fast so the PE warm-up can
        # start while the input DMAs stream ----
        masks.make_identity(nc, identb[:])

        # ---- input DMAs (SWDGE: fp32 -> fp16 cast). krw/krh go first (tiny
        # transfers; the SWDGE queue is FIFO and anything after 3MB of x
        # would land ~20us in). x loads q cols first. ----
        nc.gpsimd.dma_start(out=krw_sb[:], in_=krw_d[:])
        nc.gpsimd.dma_start(out=krh_sb[:], in_=krh_d[:])
        xv = x_sb[:].rearrange("p (c g) -> p c g", c=8, g=768)
        for col0 in (0, 256, 512):              # q, k, v column groups
            for half, src_d in ((0, xa_d), (1, xb_d)):
                nc.gpsimd.dma_start(
                    out=xv[:, 4 * half:4 * half + 4, col0:col0 + 256],
                    in_=src_d[:].rearrange("(c p) g -> p c g", p=128)
                        [:, :, col0:col0 + 256],
                )
        # one-hot rows of kaug, head-0 block only: rows 32-63: [y2(k)==j],
        # rows 64-95: [x2(k)==j]; col = 32*x2 + y2. Then DMA-replicate to
        # the other 7 head blocks (log-doubling).
        nc.gpsimd.memset(kaug[32:64, 0:HW], 0.0)
        nc.gpsimd.memset(kaug[64:96, 0:HW], 0.0)
        nc.gpsimd.affine_select(
            out=kaug[32:64, 0:HW].rearrange("p (x y) -> p x y", x=32, y=32),
            in_=kaug[32:64, 0:HW].rearrange("p (x y) -> p x y", x=32, y=32),
            compare_op=mybir.AluOpType.not_equal,
            fill=1.0,
            base=0,
            pattern=[[0, 32], [-1, 32]],
            channel_multiplier=1,
        )
        nc.gpsimd.affine_select(
            out=kaug[64:96, 0:HW].rearrange("p (x y) -> p x y", x=32, y=32),
            in_=kaug[64:96, 0:HW].rearrange("p (x y) -> p x y", x=32, y=32),
            compare_op=mybir.AluOpType.not_equal,
            fill=1.0,
            base=0,
            pattern=[[-1, 32], [0, 32]],
            channel_multiplier=1,
        )
        n = HW
        while n < NH * HW:
            rep = min(n, NH * HW - n)
            nc.sync.dma_start(
                out=kaug[32:96, n:n + rep],
                in_=kaug[32:96, 0:rep],
            )
            n += rep
        # V1: ones only in col 32 of each 33-block; V chunks fill cols 0-31
        # (copies emitted below, on GPSIMD behind the pair-0 W scatter).
        v1v = v1[:].rearrange("p (h c e) -> p h c e", h=8, c=8, e=33)
        nc.gpsimd.memset(v1v[:, :, :, 32], 1.0)
        v1c = v1[:].rearrange("p (h c e) -> p c h e", h=8, c=8, e=33)

        # ================= Phase A: transposes + rel logits =================
        with ExitStack() as actx:
            psA = actx.enter_context(tc.tile_pool(name="psA", bufs=2, space="PSUM"))
            psR = actx.enter_context(tc.tile_pool(name="psR", bufs=2, space="PSUM"))

            # PE warm-up: back-to-back REAL matmuls so the HAM clock gate
            # opens (1.2 -> 2.4 GHz) while the input DMA streams in.
            # Transpose-mode does NOT count as PE-busy for HAM, so the
            # warm-up must be plain matmuls (~3.5us of sustained activity).
            wps = psA.tile([128, 512], F32, tag="warm")
            for i in range(32):
                nc.tensor.matmul(
                    out=wps[:, 0:128], lhsT=identb[:], rhs=identb[:],
                    start=True, stop=True,
                )
            nc.vector.memset(krwT[:], 0.0)
            nc.vector.memset(krhT[:], 0.0)

            # qT: PE-transpose 4-head groups, cast, partition-scatter
            # DMAs into qaug rows 0-31. (k groups done after w_pair(0).)
            def transpose_group(kind, g, dstt):
                col0 = 256 * kind + 128 * g
                pt = psA.tile([128, HW], FP16, tag="tps")
                for c in range(8):
                    nc.tensor.transpose(
                        out=pt[:, 128 * c:128 * c + 128],
                        in_=x_sb[:, 768 * c + col0:768 * c + col0 + 128],
                        identity=identb[:, 0:128],
                    )
                scr = tscr[:, HW * g:HW * g + HW]
                # casts split across DVE/ACT so the two groups overlap
                if g == 0:
                    nc.vector.tensor_copy(out=scr, in_=pt[:])
                else:
                    nc.scalar.copy(out=scr, in_=pt[:])
                for hh in range(4):
                    h = 4 * g + hh
                    dma_eng = nc.sync if hh % 2 == 0 else nc.scalar
                    dma_eng.dma_start(
                        out=dstt[0:32, HW * h:HW * h + HW],
                        in_=tscr[32 * hh:32 * hh + 32, HW * g:HW * g + HW],
                    )

            for g in range(2):
                transpose_group(0, g, qaug)

            # key_rel transposes: krw [63,32] -> krwT cols 32-94 (zero pad).
            for srct, dst, off in ((krw_sb, krwT, 32), (krh_sb, krhT, 64)):
                pt = psA.tile([128, HW], FP16, tag="tps")
                nc.tensor.transpose(
                    out=pt[0:32, 0:63], in_=srct[:], identity=identb[0:63, 0:63]
                )
                nc.vector.tensor_copy(out=dst[0:32, off:off + 63], in_=pt[0:32, 0:63])

            qa4 = qaug[0:32, :].rearrange("p (h x y) -> p h x y", h=8, x=32, y=32)
            hdst = qaug[64:96, :].rearrange("p (h x y) -> p h x y", h=8, x=32, y=32)
            wdst = qaug[32:64, :].rearrange("p (h x y) -> p h x y", h=8, x=32, y=32)
            # wnat per head-pair block: cols 8192p + (i32, h2, x32)
            wnp = wnat[32:64, :].rearrange(
                "p (pp i h x) -> p pp i h x", pp=4, i=32, h=2, x=32)

            def w_half(p, ss, eng, pool, tag):
                # W rel for heads 2p,2p+1, y=i in [16ss, 16ss+16): one MM
                # (N=64) per i; staged contiguously (i,h,x) on DVE/ACT.
                # (runs-of-1 strided copies cost ~4.7us on DVE/ACT --
                # measured -- so the final scatter stays on GPSIMD.)
                pw = pool.tile([96, HW], F32, tag=tag)
                pwmm = pw[0:64, :].rearrange(
                    "p (i h x) -> p i h x", i=16, h=2, x=32)
                for ii in range(16):
                    i = 16 * ss + ii
                    nc.tensor.matmul(
                        out=pwmm[:, ii, :, :],
                        lhsT=krwT[:, 31 - i:95 - i],
                        rhs=qa4[:, 2 * p:2 * p + 2, :, i],
                        start=True, stop=True,
                    )
                src = pw[32:64, :].rearrange(
                    "p (i h x) -> p i h x", i=16, h=2, x=32)
                dst = wnp[:, p, 16 * ss:16 * ss + 16, :, :]
                if eng == "act":
                    nc.scalar.copy(out=dst, in_=src)
                else:
                    nc.vector.tensor_copy(out=dst, in_=src)

            def w_scatter(p, ss):
                # scatter one staged y-half to qaug rows 32-63 (runs of 1)
                # on GPSIMD cores 2-3, which own partitions 32-63.
                for hh in range(2):
                    h = 2 * p + hh
                    nc.gpsimd.tensor_copy(
                        out=wdst[:, h, :, 16 * ss:16 * ss + 16],
                        in_=wnp[:, p, 16 * ss:16 * ss + 16, hh, :]
                            .rearrange("p i x -> p x i"),
                    )

            def w_pair(p, eng, pool, tag="rel"):
                for ss in range(2):
                    w_half(p, ss, eng, pool, tag)
                for ss in range(2):
                    w_scatter(p, ss)

            def h_group(g):
                # H rel, all heads, x = 4g..4g+3; copies go straight to
                # qaug rows 64-95 (runs of 32), alternating ScalarE/DVE.
                ph = psR.tile([96, HW], F32, tag="rel")
                phmm = ph[:].rearrange("p (i h y) -> p i h y", i=4, h=8, y=32)
                for j in range(4):
                    i = 4 * g + j
                    nc.tensor.matmul(
                        out=phmm[:, j, :, :],
                        lhsT=krhT[:, 31 - i:127 - i],
                        rhs=qa4[:, :, i, :],
                        start=True, stop=True,
                    )
                dst = hdst[:, :, 4 * g:4 * g + 4, :]
                src = ph[64:96, :].rearrange(
                    "p (i h y) -> p h i y", i=4, h=8, y=32)
                if g % 2 == 0:
                    nc.scalar.copy(out=dst, in_=src)
                else:
                    nc.vector.tensor_copy(out=dst, in_=src)

            def v_copy(c):
                nc.gpsimd.tensor_copy(
                    out=v1c[:, c, :, 0:32],
                    in_=x_sb[:, 768 * c + 512:768 * c + 512 + 256]
                        .rearrange("p (h e) -> p h e", h=8),
                )

            # W pair 0, both kT groups, and ALL H groups complete first
            # (every pair needs every H stage); later W pairs + V copies
            # fill in behind pair 0's attention on their engines' queues.
            w_pair(0, "act", psR)
            transpose_group(1, 0, kaug)
            transpose_group(1, 1, kaug)
            for g in range(8):
                h_group(g)
            for c in range(8):
                v_copy(c)
            w_pair(1, "dve", psR)
            w_pair(2, "act", psR)
            w_pair(3, "dve", psR)

        # ================= Phase B: attention per head-pair =================
        with ExitStack() as bctx:
            psS = bctx.enter_context(tc.tile_pool(name="psS", bufs=3, space="PSUM"))
            psT = bctx.enter_context(tc.tile_pool(name="psT", bufs=1, space="PSUM"))
            sbW = bctx.enter_context(tc.tile_pool(name="sbW", bufs=4))
            sbE = bctx.enter_context(tc.tile_pool(name="sbE", bufs=4))
            sbA = bctx.enter_context(tc.tile_pool(name="sbA", bufs=2))
            sbT = bctx.enter_context(tc.tile_pool(name="sbT", bufs=2))
            sbR = bctx.enter_context(tc.tile_pool(name="sbR", bufs=2))

            for hp in range(NH // 2):
                # two heads share one att psum: head 2hp at partitions 0-32,
                # head 2hp+1 at partitions 64-96 (col-tiled concurrent AV).
                # Chunk loop software-pipelined: QK(c)+exp-issue(c), then the
                # pending DVE TT from (c-1), then AV(c-1) -- so the in-order
                # PE never sits behind an exp of its own chunk, and the DVE
                # never idles between TS1 and its TT (GPSIMD shift overlaps).
                att = psT.tile([97, HW], F32, tag="att")
                wexp_prev = None
                pend_tt = []
                for c in range(9):
                    wexps = []
                    if c < 8:
                        for hh in range(2):
                            h = 2 * hp + hh
                            s_ps = psS.tile([128, HW], F32, tag="sT")
                            for e in range(2):
                                nc.tensor.matmul(
                                    out=s_ps[:, 512 * e:512 * e + 512],
                                    lhsT=kaug[:, HW * h + 128 * c:HW * h + 128 * c + 128],
                                    rhs=qaug[:, HW * h + 512 * e:HW * h + 512 * e + 512],
                                    start=True, stop=True,
                                )
                            wexp = sbW.tile([128, HW], FP16, tag="wexp")
                            if (hh, c) in DVE_CHUNKS[hp]:
                                e1 = sbE.tile([128, HW], I16, tag="e1")
                                e2 = sbE.tile([128, HW], I16, tag="e2")
                                nc.vector.tensor_scalar(
                                    out=e1[:], in0=s_ps[:],
                                    scalar1=SCH_A, scalar2=SCH_B1,
                                    op0=MULT, op1=ADD,
                                )
                                nc.vector.tensor_scalar_add(
                                    out=e2[:], in0=e1[:], scalar1=-512.0,
                                )
                                pend_tt.append((e1, e2, wexp))
                            else:
                                nc.scalar.activation(
                                    out=wexp[:], in_=s_ps[:], func=EXP, scale=QSCALE,
                                )
                            wexps.append(wexp)
                    # pending TTs from the previous slot: run on DVE while
                    # this slot's QKs stream on the PE, ahead of AV(c-1).
                    for e1, e2, wexp in pend_tt:
                        nc.vector.tensor_tensor(
                            out=wexp[:],
                            in0=e1[:].bitcast(FP16),
                            in1=e2[:].bitcast(FP16),
                            op=MULT,
                        )
                    pend_tt = []
                    if c > 0:
                        # AV(c-1), e-major: the two heads' matmuls sit in
                        # different PE col groups and run concurrently.
                        for e in range(2):
                            for hh in range(2):
                                h = 2 * hp + hh
                                nc.tensor.matmul(
                                    out=att[64 * hh:64 * hh + 33, 512 * e:512 * e + 512],
                                    lhsT=v1[:, 264 * h + 33 * (c - 1):264 * h + 33 * (c - 1) + 33],
                                    rhs=wexp_prev[hh][:, 512 * e:512 * e + 512],
                                    start=(c - 1 == 0), stop=(c - 1 == 7),
                                )
                    wexp_prev = wexps

                # Output, pipelined per 512-col (e) half:
                # att -> fp16 SBUF (x 2^-6, cancels in the normalization);
                # ACT takes e=0, DVE takes e=1 so the halves overlap and the
                # (single-buffered) att psum frees for the next pair ASAP.
                att_sb = sbA.tile([112, HW], FP16, tag="attsb")
                nc.vector.memset(att_sb[96:112, :], 0.0)
                nc.scalar.activation(
                    out=att_sb[0:97, 0:512], in_=att[:, 0:512],
                    func=COPY, scale=float(2.0 ** -6),
                )
                nc.vector.tensor_scalar_mul(
                    out=att_sb[0:97, 512:1024],
                    in0=att[:, 512:1024],
                    scalar1=float(2.0 ** -6),
                )
                # xbar DMA transpose per half: [112, 512] -> [128, (c4, j112)]
                att_t = sbT.tile([128, 8 * 112], FP16, tag="attt")
                att_tv = att_t[:].rearrange("p (c j) -> p c j", c=8, j=112)
                rc = sbR.tile([128, 16], F32, tag="rc")
                rcv = rc[:].rearrange("p (c h) -> p c h", c=8, h=2)
                last = hp == NH // 2 - 1
                for e in range(2):
                    nc.sync.dma_start_transpose(
                        out=att_tv[:, 4 * e:4 * e + 4, :],
                        in_=att_sb[:, 512 * e:512 * e + 512],
                    )
                    # reciprocal of the denominators (row 32 / 96 of att)
                    nc.vector.reciprocal(
                        out=rcv[:, 4 * e:4 * e + 4, :],
                        in_=att_tv[:, 4 * e:4 * e + 4, 32:97:64],
                    )
                    for cc in range(4):
                        cg = 4 * e + cc
                        for hh in range(2):
                            h = 2 * hp + hh
                            dst = out_sb[:, 256 * cg + 32 * h:256 * cg + 32 * h + 32]
                            src = att_tv[:, cg, 64 * hh:64 * hh + 32]
                            sc = rc[:, 2 * cg + hh:2 * cg + hh + 1]
                            # ACT per-op overhead is ~2.7x DVE's here; on the
                            # last pair every scale is tail-serial, so keep
                            # them all on the (faster per-op) DVE.
                            if hh == 0 or last:
                                nc.vector.tensor_scalar_mul(
                                    out=dst, in0=src, scalar1=sc)
                            else:
                                nc.scalar.activation(
                                    out=dst, in_=src, func=COPY, scale=sc)
                # stream this pair's output columns to DRAM
                nc.sync.dma_start(
                    out=out_d[:].rearrange("(c p) d -> p c d", p=128)
                        [:, :, 64 * hp:64 * hp + 64],
                    in_=out_sb[:].rearrange("p (c d) -> p c d", c=8)
                        [:, :, 64 * hp:64 * hp + 64],
                )
    if not nc.is_finalized():
        nc.finalize()
    return nc


_NC = None


def _ensure_axon_hooks_module():
    """bass_utils imports antenv.axon_hooks unconditionally when trace=True;
    this image's antenv lacks it. Provide a stub so tracing degrades to
    no-trace instead of crashing (a real hook can be set by a profiler)."""
    import types

    if "antenv.axon_hooks" in sys.modules:
        return
    try:
        import antenv.axon_hooks  # noqa: F401
        return
    except ImportError:
        pass
    try:
        import antenv
    except ImportError:
        return
    m = types.ModuleType("antenv.axon_hooks")
    m._hook = None
    m.get_axon_ntff_profile_hook = lambda: m._hook
    m.set_axon_ntff_profile_hook = lambda h: setattr(m, "_hook", h)
    sys.modules["antenv.axon_hooks"] = m
    antenv.axon_hooks = m


def kernel(**inputs):
    global _NC
    x = np.ascontiguousarray(np.asarray(inputs["inputs"], dtype=np.float32))
    krw = np.ascontiguousarray(np.asarray(inputs["key_rel_w"], dtype=np.float32))
    krh = np.ascontiguousarray(np.asarray(inputs["key_rel_h"], dtype=np.float32))
    assert x.shape == (8, 32, 32, 768), x.shape
    assert int(inputs["dk"]) == 256 and int(inputs["dv"]) == 256
    assert int(inputs["Nh"]) == 8

    if _NC is None:
        _NC = build_nc()
    _ensure_axon_hooks_module()
    from concourse.bass_utils import run_bass_kernel_spmd

    in_maps = [
        {
            "xa": x[b].reshape(HW, CH)[:HW // 2],
            "xb": x[b].reshape(HW, CH)[HW // 2:],
            "krw": krw,
            "krh": krh,
        }
        for b in range(8)
    ]
    res = run_bass_kernel_spmd(_NC, in_maps, list(range(8)))
    kernel.last_result = res
    out = np.stack([res.results[b]["out"].reshape(32, 32, 256) for b in range(8)], 0)
    return out


if __name__ == "__main__":
    nc = build_nc()
    print("built ok")


# revision 20
# speedup vs baseline: 1.2222x; 1.0049x over previous
"""Trainium2 Bass kernel for nn_AttentionAugmentation.

Attention with 2D relative-position logits. B=8, H=W=32, dk=dv=256, Nh=8.
Sharding: data-parallel over batch (one batch per NeuronCore, 8 cores).

Per-core v3 (one batch, 8 heads of 1024x1024 attention, dkh=32):
  - fp16 datapath end-to-end (inputs cast fp32->fp16 in the SWDGE DMA):
    q/k/v, qaug/kaug, rel keys, wexp are all fp16 -- higher precision than
    bf16 buys error budget for the cheap DVE exp below.
  - qT/kT via PE transposes -> cast -> partition-scatter DMAs into
    qaug/kaug rows 0-31. dk^-0.5 folded into the exp (scale / Schraudolph A).
  - rel logits in a 96-row augmented contraction: kaug rows 32-63 one-hot
    of key y2, rows 64-95 one-hot of key x2; qaug rows 32-63 = WRELT,
    rows 64-95 = HRELT, computed by shifted krw/krh^T-window matmuls.
    Rel psum is copied STRAIGHT to qaug with strided 1x DVE/ACT copies
    (runs-of-1 for W; runs-of-32 for H) -- no staging, no GPSIMD scatter.
  - attention per head-pair, software-pipelined per 128-key chunk:
    S^T = QK matmuls (f32 psum); exp split between ACT (scalar.activation
    Exp -> fp16) and a DVE fp16 pair-product Schraudolph: one
    tensor_scalar makes i1 = int16(A*logit + B1); GPSIMD shifts
    i2 = i1 - 512 (half-period stagger); one DVE tensor_tensor multiplies
    the two int16-bitcast-fp16 staircases => exp within ~1% with ~zero
    mean bias (C=58 debias), so mixing with ACT chunks is safe.
    AV uses lhsT=[V | 1] per head; the two heads of a pair write att psum
    partitions 0-32 / 64-96 (col-tiled concurrent matmuls).
  - output per pair, pipelined per 512-col half: att psum -> fp16 SBUF
    (x2^-6; ACT half / DVE half in parallel), xbar DMA-transpose per half,
    reciprocal per half, then per-(head,chunk) reciprocal-scaled copies
    into out_sb (GPSIMD for pairs 0-2; DVE/ACT split on the last pair to
    shorten the tail), and a per-pair DMA of the 64 output channels.
"""
import sys

sys.path.insert(0, "/opt/trn_rl_repo")

from contextlib import ExitStack

import numpy as np

import concourse.bass as bass
from concourse import bacc
import concourse.mybir as mybir
from concourse import masks
from concourse.tile import TileContext

HW = 1024
CH = 768
NH = 8
F32 = mybir.dt.float32
FP16 = mybir.dt.float16
I16 = mybir.dt.int16
EXP = mybir.ActivationFunctionType.Exp
COPY = mybir.ActivationFunctionType.Copy
MULT = mybir.AluOpType.mult
ADD = mybir.AluOpType.add
QSCALE = float((256 / 8) ** -0.5)
# fp16 pair-product Schraudolph: i1 = int16(A*x + B1), i2 = i1 - 512,
# exp(x*QSCALE) ~= fp16(i1) * fp16(i2). A folds QSCALE; C=58 zeroes the
# mean bias so ACT-exp and DVE-exp chunks can mix inside one softmax row.
SCH_A = 512.0 * QSCALE / np.log(2.0)
SCH_B1 = 15360.0 + 256.0 - 58.0
# (hh, c) chunks computed on the DVE path, per pair index.
DVE_CHUNKS = {
    0: {(0, 4), (0, 6), (1, 5), (1, 7)},
    1: {(0, 1), (0, 4), (0, 6), (1, 2), (1, 5), (1, 7)},
    2: {(0, 1), (0, 4), (0, 6), (1, 2), (1, 5), (1, 7)},
    3: {(0, 1), (0, 4), (0, 6), (1, 2), (1, 5), (1, 7)},
}


def build_nc():
    nc = bacc.Bacc()
    # input split in two halves: a single [1024, 768] parameter makes the
    # axon-pjrt reshard program's dynamic-slice exceed a 16-bit semaphore
    # field in neuronx-cc (25MB concat across 8 cores), crashing walrus.
    xa_d = nc.declare_dram_parameter("xa", [HW // 2, CH], F32, isOutput=False)
    xb_d = nc.declare_dram_parameter("xb", [HW // 2, CH], F32, isOutput=False)
    krw_d = nc.declare_dram_parameter("krw", [63, 32], F32, isOutput=False)
    krh_d = nc.declare_dram_parameter("krh", [63, 32], F32, isOutput=False)
    out_d = nc.declare_dram_parameter("out", [HW, 256], F32, isOutput=True)

    with ExitStack() as octx:
        tc = octx.enter_context(TileContext(nc))
        sb = octx.enter_context(tc.tile_pool(name="persist", bufs=1))

        x_sb = sb.tile([128, 8 * CH], FP16)     # natural input: part p, col 768c+ch
        qaug = sb.tile([96, NH * HW], FP16)     # per head h: cols 1024h + (32x + y)
        kaug = sb.tile([96, NH * HW], FP16)
        v1 = sb.tile([128, NH * 8 * 33], FP16)  # per (h,c): 33 cols = V chunk | ones
        tscr = sb.tile([128, 2 * HW], FP16)     # transpose scratch (2 groups live)
        wnat = sb.tile([64, NH * HW], FP16)     # rows 32-63: W rel, (y, h, x) major
        out_sb = sb.tile([128, 8 * 256], F32)   # col 256c + ch
        identb = sb.tile([128, 128], FP16)
        krw_sb = sb.tile([63, 32], FP16)
        krh_sb = sb.tile([63, 32], FP16)
        # zero-padded transposed rel keys: walrus rejects 32-contraction
        # matmuls whose psum out starts at partition 32/64, so the rel MMs
        # use wider lhsT windows that land the useful rows at 32-63 (W) /
        # 64-95 (H) of a base-0 psum tile instead.
        krwT = sb.tile([32, 128], FP16)   # krwT[:, 32+m] = krw^T[:, m]
        krhT = sb.tile([32, 160], FP16)   # krhT[:, 64+m] = krh^T[:, m]

        # ---- constants first: identity lands fast so the PE warm-up can
        # start while the input DMAs stream ----
        masks.make_identity(nc, identb[:])

        # ---- input DMAs (SWDGE: fp32 -> fp16 cast). krw/krh go first (tiny
        # transfers; the SWDGE queue is FIFO and anything after 3MB of x
        # would land ~20us in). x loads q cols first. ----
        nc.gpsimd.dma_start(out=krw_sb[:], in_=krw_d[:])
        nc.gpsimd.dma_start(out=krh_sb[:], in_=krh_d[:])
        xv = x_sb[:].rearrange("p (c g) -> p c g", c=8, g=768)
        for col0 in (0, 256, 512):              # q, k, v column groups
            for half, src_d in ((0, xa_d), (1, xb_d)):
                nc.gpsimd.dma_start(
                    out=xv[:, 4 * half:4 * half + 4, col0:col0 + 256],
                    in_=src_d[:].rearrange("(c p) g -> p c g", p=128)
                        [:, :, col0:col0 + 256],
                )
        # one-hot rows of kaug, head-0 block only: rows 32-63: [y2(k)==j],
        # rows 64-95: [x2(k)==j]; col = 32*x2 + y2. Then DMA-replicate to
        # the other 7 head blocks (log-doubling).
        nc.gpsimd.memset(kaug[32:64, 0:HW], 0.0)
        nc.gpsimd.memset(kaug[64:96, 0:HW], 0.0)
        nc.gpsimd.affine_select(
            out=kaug[32:64, 0:HW].rearrange("p (x y) -> p x y", x=32, y=32),
            in_=kaug[32:64, 0:HW].rearrange("p (x y) -> p x y", x=32, y=32),
            compare_op=mybir.AluOpType.not_equal,
            fill=1.0,
            base=0,
            pattern=[[0, 32], [-1, 32]],
            channel_multiplier=1,
        )
        nc.gpsimd.affine_select(
            out=kaug[64:96, 0:HW].rearrange("p (x y) -> p x y", x=32, y=32),
            in_=kaug[64:96, 0:HW].rearrange("p (x y) -> p x y", x=32, y=32),
            compare_op=mybir.AluOpType.not_equal,
            fill=1.0,
            base=0,
            pattern=[[-1, 32], [0, 32]],
            channel_multiplier=1,
        )
        n = HW
        while n < NH * HW:
            rep = min(n, NH * HW - n)
            nc.sync.dma_start(
                out=kaug[32:96, n:n + rep],
                in_=kaug[32:96, 0:rep],
            )
            n += rep
        # V1: ones only in col 32 of each 33-block; V chunks fill cols 0-31
        # (copies emitted below, on GPSIMD behind the pair-0 W scatter).
        v1v = v1[:].rearrange("p (h c e) -> p h c e", h=8, c=8, e=33)
        nc.gpsimd.memset(v1v[:, :, :, 32], 1.0)
        v1c = v1[:].rearrange("p (h c e) -> p c h e", h=8, c=8, e=33)

        # ================= Phase A: transposes + rel logits =================
        with ExitStack() as actx:
            psA = actx.enter_context(tc.tile_pool(name="psA", bufs=2, space="PSUM"))
            psR = actx.enter_context(tc.tile_pool(name="psR", bufs=2, space="PSUM"))

            # PE warm-up: back-to-back REAL matmuls so the HAM clock gate
            # opens (1.2 -> 2.4 GHz) while the input DMA streams in.
            # Transpose-mode does NOT count as PE-busy for HAM, so the
            # warm-up must be plain matmuls (~3.5us of sustained activity).
            wps = psA.tile([128, HW], FP16, tag="tps")
            wpsf = wps[:].bitcast(F32)
            for i in range(32):
                nc.tensor.matmul(
                    out=wpsf[:, 0:128], lhsT=identb[:], rhs=identb[:],
                    start=True, stop=True,
                )
            nc.vector.memset(krwT[:], 0.0)
            nc.vector.memset(krhT[:], 0.0)

            # qT: PE-transpose 4-head groups, cast, partition-scatter
            # DMAs into qaug rows 0-31. (k groups done after w_pair(0).)
            def transpose_group(kind, g, dstt):
                col0 = 256 * kind + 128 * g
                pt = psA.tile([128, HW], FP16, tag="tps")
                for c in range(8):
                    nc.tensor.transpose(
                        out=pt[:, 128 * c:128 * c + 128],
                        in_=x_sb[:, 768 * c + col0:768 * c + col0 + 128],
                        identity=identb[:, 0:128],
                    )
                scr = tscr[:, HW * g:HW * g + HW]
                # casts split across DVE/ACT so the two groups overlap
                if g == 0:
                    nc.vector.tensor_copy(out=scr, in_=pt[:])
                else:
                    nc.scalar.copy(out=scr, in_=pt[:])
                for hh in range(4):
                    h = 4 * g + hh
                    dma_eng = nc.sync if hh % 2 == 0 else nc.scalar
                    dma_eng.dma_start(
                        out=dstt[0:32, HW * h:HW * h + HW],
                        in_=tscr[32 * hh:32 * hh + 32, HW * g:HW * g + HW],
                    )

            for g in range(2):
                transpose_group(0, g, qaug)

            # key_rel transposes: krw [63,32] -> krwT cols 32-94 (zero pad).
            for srct, dst, off in ((krw_sb, krwT, 32), (krh_sb, krhT, 64)):
                pt = psA.tile([128, HW], FP16, tag="tps")
                nc.tensor.transpose(
                    out=pt[0:32, 0:63], in_=srct[:], identity=identb[0:63, 0:63]
                )
                nc.vector.tensor_copy(out=dst[0:32, off:off + 63], in_=pt[0:32, 0:63])

            qa4 = qaug[0:32, :].rearrange("p (h x y) -> p h x y", h=8, x=32, y=32)
            hdst = qaug[64:96, :].rearrange("p (h x y) -> p h x y", h=8, x=32, y=32)
            wdst = qaug[32:64, :].rearrange("p (h x y) -> p h x y", h=8, x=32, y=32)
            # wnat per 4-head group: cols 4096g + (i32, h4, x32)
            wnp = wnat[32:64, :].rearrange(
                "p (g i h x) -> p g i h x", g=2, i=32, h=4, x=32)

            def w_group(g):
                # W rel for heads 4g..4g+3: one MM per y=i (N=128); 8 i per
                # psum tile; staged contiguously (i,h,x) on DVE/ACT.
                # (runs-of-1 strided copies cost ~4.7us on DVE/ACT --
                # measured -- so the final scatter stays on GPSIMD.)
                for ss in range(4):
                    pw = psR.tile([64, HW], F32, tag="rel")
                    pwmm = pw[:].rearrange(
                        "p (i h x) -> p i h x", i=8, h=4, x=32)
                    for ii in range(8):
                        i = 8 * ss + ii
                        nc.tensor.matmul(
                            out=pwmm[:, ii, :, :],
                            lhsT=krwT[:, 31 - i:95 - i],
                            rhs=qa4[:, 4 * g:4 * g + 4, :, i],
                            start=True, stop=True,
                        )
                    src = pw[32:64, :].rearrange(
                        "p (i h x) -> p i h x", i=8, h=4, x=32)
                    dst = wnp[:, g, 8 * ss:8 * ss + 8, :, :]
                    if ss % 2 == 0:
                        nc.scalar.copy(out=dst, in_=src)
                    else:
                        nc.vector.tensor_copy(out=dst, in_=src)

            def w_scatter(h):
                # scatter one head to qaug rows 32-63 (runs of 1) on
                # GPSIMD cores 2-3, which own partitions 32-63.
                nc.gpsimd.tensor_copy(
                    out=wdst[:, h, :, :],
                    in_=wnp[:, h // 4, :, h % 4, :].rearrange("p i x -> p x i"),
                )

            def h_group(g):
                # H rel, all heads, x = 4g..4g+3; copies go straight to
                # qaug rows 64-95 (runs of 32), alternating ScalarE/DVE.
                ph = psR.tile([96, HW], F32, tag="rel")
                phmm = ph[:].rearrange("p (i h y) -> p i h y", i=4, h=8, y=32)
                for j in range(4):
                    i = 4 * g + j
                    nc.tensor.matmul(
                        out=phmm[:, j, :, :],
                        lhsT=krhT[:, 31 - i:127 - i],
                        rhs=qa4[:, :, i, :],
                        start=True, stop=True,
                    )
                dst = hdst[:, :, 4 * g:4 * g + 4, :]
                src = ph[64:96, :].rearrange(
                    "p (i h y) -> p h i y", i=4, h=8, y=32)
                if g % 2 == 0:
                    nc.scalar.copy(out=dst, in_=src)
                else:
                    nc.vector.tensor_copy(out=dst, in_=src)

            def v_copy(c):
                nc.gpsimd.tensor_copy(
                    out=v1c[:, c, :, 0:32],
                    in_=x_sb[:, 768 * c + 512:768 * c + 512 + 256]
                        .rearrange("p (h e) -> p h e", h=8),
                )

            # W heads 0-3, both kT groups, and ALL H groups complete first
            # (every pair needs every H stage); W heads 4-7 + V copies
            # fill in behind pair 0/1's attention on their engines' queues.
            # GPSIMD queue order: scatters h0-h1 (pair 0), v copies (pair-0
            # AV), h2-h3 (pair 1), then the late group's h4-h7.
            w_group(0)
            transpose_group(1, 0, kaug)
            transpose_group(1, 1, kaug)
            w_scatter(0)
            w_scatter(1)
            for g in range(8):
                h_group(g)
            for c in range(8):
                v_copy(c)
            w_scatter(2)
            w_scatter(3)
            w_group(1)
            for h in range(4, 8):
                w_scatter(h)

        # ================= Phase B: attention per head-pair =================
        with ExitStack() as bctx:
            psS = bctx.enter_context(tc.tile_pool(name="psS", bufs=3, space="PSUM"))
            psT = bctx.enter_context(tc.tile_pool(name="psT", bufs=1, space="PSUM"))
            sbW = bctx.enter_context(tc.tile_pool(name="sbW", bufs=4))
            sbE = bctx.enter_context(tc.tile_pool(name="sbE", bufs=4))
            sbA = bctx.enter_context(tc.tile_pool(name="sbA", bufs=2))
            sbT = bctx.enter_context(tc.tile_pool(name="sbT", bufs=2))
            sbR = bctx.enter_context(tc.tile_pool(name="sbR", bufs=2))

            for hp in range(NH // 2):
                # two heads share one att psum: head 2hp at partitions 0-32,
                # head 2hp+1 at partitions 64-96 (col-tiled concurrent AV).
                # Chunk loop software-pipelined: QK(c)+exp-issue(c), then the
                # pending DVE TT from (c-1), then AV(c-1) -- so the in-order
                # PE never sits behind an exp of its own chunk, and the DVE
                # never idles between TS1 and its TT (GPSIMD shift overlaps).
                att = psT.tile([97, HW], F32, tag="att")
                wexp_prev = None
                pend_tt = []
                for c in range(9):
                    wexps = []
                    if c < 8:
                        for hh in range(2):
                            h = 2 * hp + hh
                            s_ps = psS.tile([128, HW], F32, tag="sT")
                            for e in range(2):
                                nc.tensor.matmul(
                                    out=s_ps[:, 512 * e:512 * e + 512],
                                    lhsT=kaug[:, HW * h + 128 * c:HW * h + 128 * c + 128],
                                    rhs=qaug[:, HW * h + 512 * e:HW * h + 512 * e + 512],
                                    start=True, stop=True,
                                )
                            wexp = sbW.tile([128, HW], FP16, tag="wexp")
                            if (hh, c) in DVE_CHUNKS[hp]:
                                e1 = sbE.tile([128, HW], I16, tag="e1")
                                e2 = sbE.tile([128, HW], I16, tag="e2")
                                nc.vector.tensor_scalar(
                                    out=e1[:], in0=s_ps[:],
                                    scalar1=SCH_A, scalar2=SCH_B1,
                                    op0=MULT, op1=ADD,
                                )
                                nc.vector.tensor_scalar_add(
                                    out=e2[:], in0=e1[:], scalar1=-512.0,
                                )
                                pend_tt.append((e1, e2, wexp))
                            else:
                                nc.scalar.activation(
                                    out=wexp[:], in_=s_ps[:], func=EXP, scale=QSCALE,
                                )
                            wexps.append(wexp)
                    # pending TTs from the previous slot: run on DVE while
                    # this slot's QKs stream on the PE, ahead of AV(c-1).
                    for e1, e2, wexp in pend_tt:
                        nc.vector.tensor_tensor(
                            out=wexp[:],
                            in0=e1[:].bitcast(FP16),
                            in1=e2[:].bitcast(FP16),
                            op=MULT,
                        )
                    pend_tt = []
                    if c > 0:
                        # AV(c-1), e-major: the two heads' matmuls sit in
                        # different PE col groups and run concurrently.
                        for e in range(2):
                            for hh in range(2):
                                h = 2 * hp + hh
                                nc.tensor.matmul(
                                    out=att[64 * hh:64 * hh + 33, 512 * e:512 * e + 512],
                                    lhsT=v1[:, 264 * h + 33 * (c - 1):264 * h + 33 * (c - 1) + 33],
                                    rhs=wexp_prev[hh][:, 512 * e:512 * e + 512],
                                    start=(c - 1 == 0), stop=(c - 1 == 7),
                                )
                    wexp_prev = wexps

                # Output, pipelined per 512-col (e) half:
                # att -> fp16 SBUF (x 2^-6, cancels in the normalization);
                # ACT takes e=0, DVE takes e=1 so the halves overlap and the
                # (single-buffered) att psum frees for the next pair ASAP.
                att_sb = sbA.tile([112, HW], FP16, tag="attsb")
                nc.vector.memset(att_sb[96:112, :], 0.0)
                nc.scalar.activation(
                    out=att_sb[0:97, 0:512], in_=att[:, 0:512],
                    func=COPY, scale=float(2.0 ** -6),
                )
                nc.vector.tensor_scalar_mul(
                    out=att_sb[0:97, 512:1024],
                    in0=att[:, 512:1024],
                    scalar1=float(2.0 ** -6),
                )
                # xbar DMA transpose per half: [112, 512] -> [128, (c4, j112)]
                att_t = sbT.tile([128, 8 * 112], FP16, tag="attt")
                att_tv = att_t[:].rearrange("p (c j) -> p c j", c=8, j=112)
                rc = sbR.tile([128, 16], F32, tag="rc")
                rcv = rc[:].rearrange("p (c h) -> p c h", c=8, h=2)
                last = hp == NH // 2 - 1
                for e in range(2):
                    nc.sync.dma_start_transpose(
                        out=att_tv[:, 4 * e:4 * e + 4, :],
                        in_=att_sb[:, 512 * e:512 * e + 512],
                    )
                    # reciprocal of the denominators (row 32 / 96 of att)
                    nc.vector.reciprocal(
                        out=rcv[:, 4 * e:4 * e + 4, :],
                        in_=att_tv[:, 4 * e:4 * e + 4, 32:97:64],
                    )
                    for cc in range(4):
                        cg = 4 * e + cc
                        for hh in range(2):
                            h = 2 * hp + hh
                            dst = out_sb[:, 256 * cg + 32 * h:256 * cg + 32 * h + 32]
                            src = att_tv[:, cg, 64 * hh:64 * hh + 32]
                            sc = rc[:, 2 * cg + hh:2 * cg + hh + 1]
                            # ACT per-op overhead is ~2.7x DVE's here; on the
                            # last pair every scale is tail-serial, so keep
                            # them all on the (faster per-op) DVE.
                            if hh == 0 or last:
                                nc.vector.tensor_scalar_mul(
                                    out=dst, in0=src, scalar1=sc)
                            else:
                                nc.scalar.activation(
                                    out=dst, in_=src, func=COPY, scale=sc)
                # stream this pair's output columns to DRAM
                nc.sync.dma_start(
                    out=out_d[:].rearrange("(c p) d -> p c d", p=128)
                        [:, :, 64 * hp:64 * hp + 64],
                    in_=out_sb[:].rearrange("p (c d) -> p c d", c=8)
                        [:, :, 64 * hp:64 * hp + 64],
                )
    if not nc.is_finalized():
        nc.finalize()
    return nc


_NC = None


def _ensure_axon_hooks_module():
    """bass_utils imports antenv.axon_hooks unconditionally when trace=True;
    this image's antenv lacks it. Provide a stub so tracing degrades to
    no-trace instead of crashing (a real hook can be set by a profiler)."""
    import types

    if "antenv.axon_hooks" in sys.modules:
        return
    try:
        import antenv.axon_hooks  # noqa: F401
        return
    except ImportError:
        pass
    try:
        import antenv
    except ImportError:
        return
    m = types.ModuleType("antenv.axon_hooks")
    m._hook = None
    m.get_axon_ntff_profile_hook = lambda: m._hook
    m.set_axon_ntff_profile_hook = lambda h: setattr(m, "_hook", h)
    sys.modules["antenv.axon_hooks"] = m
    antenv.axon_hooks = m


def kernel(**inputs):
    global _NC
    x = np.ascontiguousarray(np.asarray(inputs["inputs"], dtype=np.float32))
    krw = np.ascontiguousarray(np.asarray(inputs["key_rel_w"], dtype=np.float32))
    krh = np.ascontiguousarray(np.asarray(inputs["key_rel_h"], dtype=np.float32))
    assert x.shape == (8, 32, 32, 768), x.shape
    assert int(inputs["dk"]) == 256 and int(inputs["dv"]) == 256
    assert int(inputs["Nh"]) == 8

    if _NC is None:
        _NC = build_nc()
    _ensure_axon_hooks_module()
    from concourse.bass_utils import run_bass_kernel_spmd

    in_maps = [
        {
            "xa": x[b].reshape(HW, CH)[:HW // 2],
            "xb": x[b].reshape(HW, CH)[HW // 2:],
            "krw": krw,
            "krh": krh,
        }
        for b in range(8)
    ]
    res = run_bass_kernel_spmd(_NC, in_maps, list(range(8)))
    kernel.last_result = res
    out = np.stack([res.results[b]["out"].reshape(32, 32, 256) for b in range(8)], 0)
    return out


if __name__ == "__main__":
    nc = build_nc()
    print("built ok")
